# revision 1
# baseline (speedup 1.0000x reference)
"""Trainium2 Bass kernel for nn_GATTrafficPredictionModel.

Mathematical collapse exploited (holds for every input by construction of the
model, not by luck of the data):
  - h = broadcast(x[:, -1, :]) makes all N=512 node features identical per
    sample, and the adjacency is dense all-ones.
  - GAT attention scores e[i,j] = leakyrelu(s_src[i] + s_dst[j]) are therefore
    constant over (i, j), so softmax over neighbors is exactly uniform (1/512,
    exact in fp32 since 512 is a power of two), and the attention-weighted sum
    of identical rows reproduces the row itself.  Both GAT layers collapse to
    per-sample linear maps; a1/a2 attention vectors drop out entirely.

Collapsed computation (B=32, F=128, K=8, H=64, C=64, N=512):
    z      = x[:, -1, :]                          (B, F)
    u      = elu(z @ W_heads)  flattened heads    (B, K*H)
    w_row  = u @ W_out                            (B, C)
    S      = sum_n Wf.reshape(C, N, C)[:, n, :]   (C, C)
    out    = w_row @ S.T + bf                     (B, C)

Sharding: the only large input is Wf (64 x 32768 fp32, 8 MiB).  Each of the 8
cores owns 8 output channels c' (8 contiguous rows of Wf, contiguous in
DRAM), reduces them to S^T[:, c'_range] on-device, and computes its disjoint
slice out^T[c'_range, :].  The tiny upstream GEMMs (u, w_row) are replicated
on every core.  Host-side work is input slicing/layout and concatenating the
eight disjoint (8, 32) output slices.

Precision: inputs are shipped to the device as fp16 (weights/activations
pack and the Wf shard); every reduction/matmul accumulates in fp32 (DVE
reduce out fp32, PE PSUM fp32).  The fp32-only aux constants (block-ones
mask + bias) ride inside the fp16 pack as raw bits and are recovered with a
bitcast view, so the kernel issues just three input DMAs.  Measured
end-to-end relative error vs the fp32 jax reference: 4.1e-4 (fp32 loads:
7e-7; fp16 halves the DMA-bound kernel's traffic).  Per-core traffic
0.71 MiB -> ~2.0 us DMA roofline at 358 GB/s HBM; measured steady-state
~2.4 us/invocation on TRN2 hardware (differential timing), TimelineSim
single-shot estimate ~10.5 us including NEFF launch/drain overheads
(~5 us structural).
"""

import numpy as np

import concourse.bass as bass
import concourse.bacc as bacc
import concourse.mybir as mybir
import concourse.tile as tile
from concourse.bass_utils import run_bass_kernel_spmd

N_CORES = 8
B, S_SEQ, F = 32, 12, 128
K, H, C, N = 8, 64, 64, 512
ROWS = C // N_CORES          # output channels per core
F32 = mybir.dt.float32
F16 = mybir.dt.float16
AF = mybir.ActivationFunctionType

# Variant knobs (tuned via TimelineSim + differential HW benching; env vars
# only for experimentation -- defaults are the shipping config)
import os
NCHUNK = int(os.environ.get("KV_NCHUNK", "2"))
WF_BUFS = int(os.environ.get("KV_WF_BUFS", "2"))
LAYOUT = os.environ.get("KV_LAYOUT", "contig")    # strided | contig
if "KV_CWS" in os.environ:   # asymmetric chunk widths, comma-separated
    CWS = [int(v) for v in os.environ["KV_CWS"].split(",")]
    assert sum(CWS) == 2048 and all(v % 32 == 0 for v in CWS)
    NCHUNK = len(CWS)
else:
    CWS = [2048 // NCHUNK] * NCHUNK
CWOFF = [sum(CWS[:i]) for i in range(NCHUNK)]


# fp16 weight-pack column layout: zt | wht | wot | negones | aux-bits
# (aux = fp32 block-ones + bias carried as raw bits in 2 fp16 cols per fp32;
#  a bitcast view recovers the fp32 AP -- saves a whole DMA, and each DMA
#  costs ~0.65us of serialized HWDGE descriptor-gen)
ZT0 = 0
WHT0 = ZT0 + B
WOT0 = WHT0 + K * H
NEG0 = WOT0 + 4 * C
AUXBITS0 = NEG0 + B          # must be even (fp32 view needs 4B alignment)
BO0 = 0                      # within the fp32 aux view
BFT0 = BO0 + ROWS
AUX_COLS = BFT0 + 1
SMALL_COLS = AUXBITS0 + 2 * AUX_COLS


def _emit_body(nc, pool, wfpool, psum, t, tc=None):
    """One full per-core computation; `t` maps dram tensor names to handles."""
    # DMA issue order on the one HWDGE ring (FIFO): wf chunk 0 first so its
    # DVE reduce starts as early as possible, then the packed small inputs
    # (the GEMM chain they gate has ~1us of slack), then the remaining wf
    # chunks.  Packing the small inputs into two DMAs matters because every
    # dma_start costs ~0.65us of serialized HWDGE descriptor-gen.
    wf_tiles = [wfpool.tile([128, CWS[ci]], F16, tag=f"wfchunk{ci}",
                            name=f"wfchunk{ci}")
                for ci in range(NCHUNK)]
    nc.sync.dma_start(wf_tiles[0][:], t["wf"][:, 0:CWS[0]])
    small_s = pool.tile([128, SMALL_COLS], F16, tag="small")
    nc.sync.dma_start(small_s[:], t["small"][:])
    for ci in range(1, NCHUNK):
        nc.sync.dma_start(wf_tiles[ci][:],
                          t["wf"][:, CWOFF[ci]:CWOFF[ci] + CWS[ci]])
    zt_s = small_s[:, ZT0:ZT0 + B]
    wht_s = small_s[:, WHT0:WHT0 + K * H]
    wot_s = small_s[:, WOT0:WOT0 + 4 * C]
    neg_s = small_s[:, NEG0:NEG0 + B]
    aux_v = small_s[:, AUXBITS0:AUXBITS0 + 2 * AUX_COLS].bitcast(F32)
    bo_s = aux_v[:, BO0:BO0 + ROWS]
    bft_s = aux_v[0:ROWS, BFT0:BFT0 + 1]

    # ---- Wf shard -> S^T[:, core slice] ------------------------------
    # wf rows hold this core's 8 Wf rows reshaped (128, 2048):
    #   wf[p, f] = Wf[c'0 + p//16, (p%16)*2048 + f]
    # layout=strided: flat column j = (p%16)*2048 + f with j = n*64 + c:
    #   n = (p%16)*32 + f//64, c = f%64  -> reduce n_lo = f//64 (stride 64).
    # layout=contig: host pre-transposes so f = c*32 + n_lo -> reduce the
    #   contiguous innermost n_lo.
    # Then reduce p%16 (n_hi) and transpose to (c, c') in one PE matmul
    # against a block-ones mask.
    if LAYOUT == "contig":
        # host layout: f = c*32 + n_lo; chunk ci covers c in [CPC*ci, ...).
        # Each chunk reduces its contiguous innermost n_lo into a disjoint
        # slice of fsum -- no tree adds.
        fsum = pool.tile([128, C], F32, tag="fsum")
        for ci in range(NCHUNK):
            c0, cn = CWOFF[ci] // 32, CWS[ci] // 32
            nc.vector.tensor_reduce(
                fsum[:, c0:c0 + cn],
                wf_tiles[ci][:].rearrange("p (c n) -> p c n", n=32),
                axis=mybir.AxisListType.X,
                op=mybir.AluOpType.add,
            )
    else:
        # dram layout f = n_lo*64 + c; reduce n_lo (stride 64) per chunk,
        # then tree-add the partials.
        parts = []
        for ci in range(NCHUNK):
            tl = wf_tiles[ci]
            part = pool.tile([128, C], F32, tag=f"part{ci}")
            nc.vector.tensor_reduce(
                part[:],
                tl[:].rearrange("p (n c) -> p c n", c=C),
                axis=mybir.AxisListType.X,
                op=mybir.AluOpType.add,
            )
            parts.append(part)
        acc = parts
        while len(acc) > 1:
            nxt = []
            for i in range(0, len(acc), 2):
                sm = pool.tile([128, C], F32, tag=f"tree{len(acc)}_{i}")
                nc.vector.tensor_add(sm[:], acc[i][:], acc[i + 1][:])
                nxt.append(sm)
            acc = nxt
        fsum = acc[0]

    # ---- u^T = elu(W_heads^T z) -------------------------------------
    wh_p = psum.tile([128, 4 * B], F32, tag="whp")
    for j in range(4):
        nc.tensor.matmul(
            wh_p[:, B * j:B * (j + 1)],
            wht_s[:, 128 * j:128 * (j + 1)],
            zt_s,
            start=True, stop=True,
        )
    # elu(x) = relu(x) + exp(-relu(-x)) - 1; the three activations stay on
    # ACT (engine-local ordering, no cross-engine sems).  The sum and the -1
    # are folded into the w_row matmul accumulation below (relu and exp fed
    # as separate rhs operands, -1 via a constant negative-ones rhs), so no
    # vector-engine combine competes with the wf reduces.
    rneg_t = pool.tile([128, 4 * B], F16, tag="rneg")
    nc.scalar.activation(rneg_t[:], wh_p[:], AF.Relu, scale=-1.0)
    exp_t = pool.tile([128, 4 * B], F16, tag="exp")
    nc.scalar.activation(exp_t[:], rneg_t[:], AF.Exp, scale=-1.0)
    relu_t = pool.tile([128, 4 * B], F16, tag="relu")
    nc.scalar.activation(relu_t[:], wh_p[:], AF.Relu)

    # ---- w_row^T = W_out^T (relu + exp - 1) -------------------------
    wr_p = psum.tile([C, B], F32, tag="wrp")
    rhs_list = []
    for j in range(4):
        rhs_list.append((j, exp_t[:, B * j:B * (j + 1)]))
        rhs_list.append((j, neg_s))
    for j in range(4):
        rhs_list.append((j, relu_t[:, B * j:B * (j + 1)]))
    for mm, (j, rhs) in enumerate(rhs_list):
        nc.tensor.matmul(
            wr_p[:],
            wot_s[:, C * j:C * (j + 1)],
            rhs,
            start=(mm == 0), stop=(mm == len(rhs_list) - 1),
        )
    wr_s = pool.tile([C, B], F32, tag="wrs")
    nc.vector.tensor_copy(wr_s[:], wr_p[:])   # DVE: idle after the reduces

    # Emitted after the wr group: PE executes in program order, and the
    # fsum-gated matmul must not precede the ready wr work.
    st_p = psum.tile([C, ROWS], F32, tag="stp")
    nc.tensor.matmul(st_p[:], fsum[:], bo_s, start=True, stop=True)
    st_s = pool.tile([C, ROWS], F32, tag="sts")
    nc.vector.tensor_copy(st_s[:], st_p[:])   # DVE: idle after the reduces

    # ---- out^T[c' slice] = S^T.T w_row^T + bf -----------------------
    o_p = psum.tile([ROWS, B], F32, tag="op")
    nc.tensor.matmul(o_p[:], st_s[:], wr_s[:], start=True, stop=True)
    o_s = pool.tile([ROWS, B], F32, tag="os")
    nc.scalar.activation(o_s[:], o_p[:], AF.Identity, bias=bft_s)
    nc.sync.dma_start(t["out"][:], o_s[:])


def _build_nc(reps=1, loop_iters=None):
    nc = bacc.Bacc("TRN2", target_bir_lowering=False, debug=False,
                   num_devices=N_CORES)

    t = {
        "wf": nc.dram_tensor("wf", [128, 2048], F16, kind="ExternalInput"),
        "small": nc.dram_tensor("small", [128, SMALL_COLS], F16,
                                kind="ExternalInput"),
        "out": nc.dram_tensor("out", [ROWS, B], F32, kind="ExternalOutput"),
    }

    with tile.TileContext(nc) as tc:
        with (
            tc.tile_pool(name="pool", bufs=int(os.environ.get("KV_POOL_BUFS", "3"))) as pool,
            tc.tile_pool(name="wfpool", bufs=WF_BUFS) as wfpool,
            tc.tile_pool(name="psum", bufs=2, space=bass.MemorySpace.PSUM) as psum,
        ):
            if loop_iters:
                tc.For_i_unrolled(
                    0, loop_iters, 1,
                    lambda iv: _emit_body(nc, pool, wfpool, psum, t, tc),
                    max_unroll=8,
                )
            else:
                for _rep in range(reps):
                    _emit_body(nc, pool, wfpool, psum, t, tc)

    nc.compile()
    return nc


_NC_CACHE = None
_last_in_maps = None


def _make_in_maps(x, W_heads, W_out, Wf, bf):
    x = np.ascontiguousarray(np.asarray(x, np.float32))
    W_heads = np.ascontiguousarray(np.asarray(W_heads, np.float32))
    W_out = np.ascontiguousarray(np.asarray(W_out, np.float32))
    Wf = np.ascontiguousarray(np.asarray(Wf, np.float32))
    bf = np.ascontiguousarray(np.asarray(bf, np.float32))

    small = np.zeros((128, SMALL_COLS), np.float16)
    small[:, ZT0:ZT0 + B] = x[:, -1, :].T                          # (128, 32)
    small[:, WHT0:WHT0 + K * H] = \
        W_heads.transpose(1, 0, 2).reshape(F, K * H)               # (128, 512)
    small[:, WOT0:WOT0 + 4 * C] = \
        W_out.reshape(4, 128, C).transpose(1, 0, 2).reshape(128, 4 * C)
    small[:, NEG0:NEG0 + B] = -1.0                                 # elu -1 fold
    aux = np.zeros((128, AUX_COLS), np.float32)
    aux[np.arange(128), BO0 + np.arange(128) // 16] = 1.0          # block-ones

    in_maps = []
    for c in range(N_CORES):
        shard = Wf[ROWS * c:ROWS * (c + 1)]
        if LAYOUT == "contig":
            # [c'_loc, n_hi, n_lo, c] -> [c'_loc, n_hi, c, n_lo]
            wf_host = np.ascontiguousarray(
                shard.reshape(ROWS, 16, 32, C).transpose(0, 1, 3, 2)
                .astype(np.float16)
            ).reshape(128, 2048)
        else:
            wf_host = np.ascontiguousarray(
                shard.reshape(128, 2048).astype(np.float16))
        aux_c = aux.copy()
        aux_c[0:ROWS, BFT0] = bf[ROWS * c:ROWS * (c + 1)]
        small_c = small.copy()
        small_c[:, AUXBITS0:AUXBITS0 + 2 * AUX_COLS] = \
            aux_c.view(np.float16)                  # fp32 bits in fp16 slots
        in_maps.append({"wf": wf_host, "small": small_c})
    return in_maps


def kernel(x, W_heads, a1_heads, a2_heads, W_out, a1_out, a2_out, Wf, bf):
    global _NC_CACHE
    if _NC_CACHE is None:
        _NC_CACHE = _build_nc()
    nc = _NC_CACHE

    in_maps = _make_in_maps(x, W_heads, W_out, Wf, bf)
    global _last_in_maps
    _last_in_maps = in_maps
    res = run_bass_kernel_spmd(nc, in_maps, list(range(N_CORES)))
    outT = np.concatenate([res.results[i]["out"] for i in range(N_CORES)], axis=0)
    return np.ascontiguousarray(outT.T)                            # (32, 64)



# revision 2
# speedup vs baseline: 1.2511x; 1.2511x over previous
"""Trainium2 Bass kernel for nn_GATTrafficPredictionModel.

Mathematical collapse exploited (holds for every input by construction of the
model, not by luck of the data):
  - h = broadcast(x[:, -1, :]) makes all N=512 node features identical per
    sample, and the adjacency is dense all-ones.
  - GAT attention scores e[i,j] = leakyrelu(s_src[i] + s_dst[j]) are therefore
    constant over (i, j), so softmax over neighbors is exactly uniform (1/512,
    exact in fp32), and the attention-weighted sum of identical rows
    reproduces the row itself.  Both GAT layers collapse to per-sample linear
    maps; a1/a2 attention vectors drop out entirely.

Collapsed computation (B=32, F=128, K=8, H=64, C=64, N=512):
    z      = x[:, -1, :]                          (B, F)
    u      = elu(z @ W_heads)  flattened heads    (B, K*H)
    w_row  = u @ W_out                            (B, C)
    S      = sum_n Wf.reshape(C, N, C)[:, n, :]   (C, C)
    out    = w_row @ S.T + bf                     (B, C)

Sharding: each of the 8 cores owns 8 output channels c' (8 contiguous rows
of Wf), reduces them to S^T[:, c'_range] on-device, and computes its disjoint
slice out^T[c'_range, :].  The tiny upstream GEMMs are replicated per core.

Optimizations over the previous (2127 ns) version:
  - Wf ships as fp8e3 (e3m4) instead of fp16 -- 256 KiB/core instead of 512.
    The quantizer uses error diffusion along n (the axis the device sums
    over): the residual of each cast is carried into the next element, so the
    *sum* of the shipped fp8 values matches the fp32 sum to within one
    quantum.  End-to-end rel err ~6e-4 (vs 1.1e-2 for naive fp8).
  - The n-reduction moves from DVE tensor_reduce (1x mode, ~2.2 us for 2048
    cols) to TensorE: matmuls against a constant block-identity mask
    contract 128 partition rows at a time at 2.4 GHz.  Only a short
    [*, 8*NI] -> [*, 8] tail reduce stays on DVE.
  - Optional 2-way column tiling (KV_MODE=coltile, default): two concurrent
    matmul streams on array column groups 0-63 / 64-127 halve the PE
    streaming time for the Wf reduction.  The two partition-halves of S^T
    are summed for free inside the final matmul by duplicating w_row^T rows
    via a 0-stride lhs access pattern.
  - elu recombined as (relu(x) - 1) + exp(-relu(-x)) with one fused DVE
    scalar_tensor_tensor; final out = (o_p * sWf) + bf is one fused DVE
    tensor_scalar with both scalars riding as per-partition data (so the
    compiled program has no input-derived immediates).

Per-core DMA: wf 256 KiB fp8 + small pack ~201 KiB fp16 = ~457 KiB
=> ~1.28 us at 358 GB/s HBM/NC.  Engine budgets (steady state, warm PE):
DMA ~1.28 us | PE ~1.1 (coltile) / ~1.6 (plain) | ACT ~1.2 | DVE ~0.7.
"""

import os
import numpy as np
import ml_dtypes

import concourse.bass as bass
import concourse.bacc as bacc
import concourse.mybir as mybir
import concourse.tile as tile
from concourse.bass_utils import run_bass_kernel_spmd

N_CORES = 8
B, S_SEQ, F = 32, 12, 128
K, H, C, N = 8, 64, 64, 512
ROWS = C // N_CORES          # output channels per core
F32 = mybir.dt.float32
F16 = mybir.dt.float16
F8E3 = mybir.dt.float8e3
AF = mybir.ActivationFunctionType
ALU = mybir.AluOpType

MODE = os.environ.get("KV_MODE", "coltile")   # coltile | plain
NI = 8 if MODE == "coltile" else 16           # psum ni-width per c'
NKCHUNK = 16                                  # accumulation steps per stream

# small-pack column layout (fp16): zt | wht | wot | aux-bits
ZT0 = 0
WHT0 = ZT0 + B
WOT0 = WHT0 + K * H
AUXBITS0 = WOT0 + 4 * C      # even => fp32 bitcast view is 4B aligned
BFT_COL = 0                  # aux fp32 col 0: bias (rows 0..ROWS)
SWF_COL = 1                  # aux fp32 col 1: Wf scale (rows 0..ROWS)
AUX_COLS = 2
SMALL_COLS = AUXBITS0 + 2 * AUX_COLS


def _emit_consts(nc, cpool, t):
    """Hoisted once per program: the block-identity reduction mask."""
    mask_s = cpool.tile([128, C], F8E3, tag="mask", name="mask")
    nc.sync.dma_start(mask_s[:], t["cst"][:])
    return mask_s


def _emit_body(nc, pool, wfpool, psum, t, mask_s, tc=None):
    """One full per-core computation; `t` maps dram tensor names to handles."""
    small_s = pool.tile([128, SMALL_COLS], F16, tag="small")
    nc.sync.dma_start(small_s[:], t["small"][:])
    wf_tiles = [wfpool.tile([128, 1024], F8E3, tag=f"wfchunk{ci}",
                            name=f"wfchunk{ci}")
                for ci in range(2)]
    for ci in range(2):
        nc.sync.dma_start(wf_tiles[ci][:], t["wf"][:, 1024 * ci:1024 * (ci + 1)])

    zt_s = small_s[:, ZT0:ZT0 + B]
    wht_s = small_s[:, WHT0:WHT0 + K * H]
    wot_s = small_s[:, WOT0:WOT0 + 4 * C]
    aux_v = small_s[:, AUXBITS0:AUXBITS0 + 2 * AUX_COLS].bitcast(F32)
    bft_s = aux_v[0:ROWS, BFT_COL:BFT_COL + 1]
    swf_s = aux_v[0:ROWS, SWF_COL:SWF_COL + 1]

    # ---- u-pre = W_heads^T z  (4 chunks of 128 kh each) ---------------
    wh_p = psum.tile([128, 4 * B], F32, tag="whp")
    for j in range(4):
        nc.tensor.matmul(
            wh_p[:, B * j:B * (j + 1)],
            wht_s[:, 128 * j:128 * (j + 1)],
            zt_s,
            start=True, stop=True,
        )

    # ---- S^T from the Wf shard: PE mask-matmul reduction --------------
    # wf col layout (plain):   j = k*128 + c'l*16 + ni   (h = k*16 + ni)
    # wf col layout (coltile): j = k*128 + T*64 + c'l*8 + ni
    #                          (h = T*128 + k*8 + ni), T = array col group
    if MODE == "coltile":
        st_p = psum.tile([128, C], F32, tag="stp")
        for k in range(NKCHUNK):
            ci, off = k // 8, (k % 8) * 128
            for T in range(2):
                nc.tensor.matmul(
                    st_p[64 * T:64 * (T + 1), :],
                    mask_s[:],
                    wf_tiles[ci][:, off + 64 * T:off + 64 * (T + 1)],
                    start=(k == 0), stop=(k == NKCHUNK - 1),
                )
        st_rows = 128
    else:
        st_p = psum.tile([C, 8 * NI], F32, tag="stp")
        for k in range(NKCHUNK):
            ci, off = k // 8, (k % 8) * 128
            nc.tensor.matmul(
                st_p[:],
                mask_s[:],
                wf_tiles[ci][:, off:off + 128],
                start=(k == 0), stop=(k == NKCHUNK - 1),
            )
        st_rows = C

    # ---- elu: u = (relu(x) - 1) + exp(-relu(-x)) ----------------------
    rneg_s = pool.tile([128, 4 * B], F16, tag="rneg")
    nc.scalar.activation(rneg_s[:], wh_p[:], AF.Relu, scale=-1.0)
    e1_s = pool.tile([128, 4 * B], F16, tag="e1")
    nc.scalar.activation(e1_s[:], rneg_s[:], AF.Exp, scale=-1.0)
    r_s = pool.tile([128, 4 * B], F16, tag="r")
    nc.scalar.activation(r_s[:], wh_p[:], AF.Relu)
    u_s = pool.tile([128, 4 * B], F16, tag="u")
    nc.vector.scalar_tensor_tensor(
        u_s[:], r_s[:], -1.0, e1_s[:], op0=ALU.add, op1=ALU.add)

    # ---- w_row^T = W_out^T u ------------------------------------------
    wr_p = psum.tile([st_rows, B], F32, tag="wrp")
    for j in range(4):
        wot_j = wot_s[:, C * j:C * (j + 1)]
        if MODE == "coltile":
            # duplicate w_row^T onto partitions 64..127 via a 0-stride
            # lhs free dim, so the final matmul's 128-partition contraction
            # sums the two S^T halves for free.
            wot_j = wot_j[:, None, :].broadcast_to([128, 2, C])
        nc.tensor.matmul(
            wr_p[:], wot_j, u_s[:, B * j:B * (j + 1)],
            start=(j == 0), stop=(j == 3),
        )
    wr_s = pool.tile([st_rows, B], F32, tag="wrs")
    nc.vector.tensor_copy(wr_s[:], wr_p[:])

    # ---- tail reduce over ni: S^T slice -------------------------------
    st_s = pool.tile([st_rows, ROWS], F32, tag="sts")
    nc.vector.tensor_reduce(
        st_s[:],
        st_p[:].rearrange("p (c n) -> p c n", n=NI),
        axis=mybir.AxisListType.X,
        op=ALU.add,
    )

    # ---- out^T[c' slice] = sWf * (S^T.T w_row^T) + bf -----------------
    o_p = psum.tile([ROWS, B], F32, tag="op")
    nc.tensor.matmul(o_p[:], st_s[:], wr_s[:], start=True, stop=True)
    o_s = pool.tile([ROWS, B], F32, tag="os")
    nc.vector.tensor_scalar(
        o_s[:], o_p[:], swf_s, bft_s, op0=ALU.mult, op1=ALU.add)
    nc.sync.dma_start(t["out"][:], o_s[:])


def _build_nc(reps=1, loop_iters=None):
    nc = bacc.Bacc("TRN2", target_bir_lowering=False, debug=False,
                   num_devices=N_CORES)

    t = {
        "wf": nc.dram_tensor("wf", [128, 2048], F8E3, kind="ExternalInput"),
        "small": nc.dram_tensor("small", [128, SMALL_COLS], F16,
                                kind="ExternalInput"),
        "cst": nc.dram_tensor("cst", [128, C], F8E3, kind="ExternalInput"),
        "out": nc.dram_tensor("out", [ROWS, B], F32, kind="ExternalOutput"),
    }

    with tile.TileContext(nc) as tc:
        with (
            tc.tile_pool(name="cpool", bufs=1) as cpool,
            tc.tile_pool(name="pool", bufs=int(os.environ.get("KV_POOL_BUFS", "3"))) as pool,
            tc.tile_pool(name="wfpool", bufs=int(os.environ.get("KV_WF_BUFS", "2"))) as wfpool,
            tc.tile_pool(name="psum", bufs=2, space=bass.MemorySpace.PSUM) as psum,
        ):
            mask_s = _emit_consts(nc, cpool, t)
            if loop_iters:
                tc.For_i_unrolled(
                    0, loop_iters, 1,
                    lambda iv: _emit_body(nc, pool, wfpool, psum, t, mask_s, tc),
                    max_unroll=8,
                )
            else:
                for _rep in range(reps):
                    _emit_body(nc, pool, wfpool, psum, t, mask_s, tc)

    nc.compile()
    return nc


_NC_CACHE = None
_last_in_maps = None


def _quant_wf_feedback(Wf):
    """fp8e3 quantization of Wf with error diffusion along n (the summed
    axis): sum_n q[:, n, :] == sum_n Wf[:, n, :] to within one quantum."""
    m = float(np.abs(Wf).max())
    swf = float(2.0 ** np.ceil(np.log2(m / 7.75))) if m > 0 else 1.0
    W = (Wf / swf).reshape(C, N, C).astype(np.float32)
    q = np.empty((C, N, C), dtype=ml_dtypes.float8_e3m4)
    carry = np.zeros((C, C), np.float32)
    for n in range(N):
        tgt = W[:, n, :] + carry
        qn = tgt.astype(ml_dtypes.float8_e3m4)
        carry = tgt - qn.astype(np.float32)
        q[:, n, :] = qn
    return q.reshape(C, N * C), swf


def _make_in_maps(x, W_heads, W_out, Wf, bf):
    x = np.ascontiguousarray(np.asarray(x, np.float32))
    W_heads = np.ascontiguousarray(np.asarray(W_heads, np.float32))
    W_out = np.ascontiguousarray(np.asarray(W_out, np.float32))
    Wf = np.ascontiguousarray(np.asarray(Wf, np.float32))
    bf = np.ascontiguousarray(np.asarray(bf, np.float32))

    small = np.zeros((128, SMALL_COLS), np.float16)
    small[:, ZT0:ZT0 + B] = x[:, -1, :].T                          # (128, 32)
    small[:, WHT0:WHT0 + K * H] = \
        W_heads.transpose(1, 0, 2).reshape(F, K * H)               # (128, 512)
    small[:, WOT0:WOT0 + 4 * C] = \
        W_out.reshape(4, 128, C).transpose(1, 0, 2).reshape(128, 4 * C)

    qWf, swf = _quant_wf_feedback(Wf)                              # (64, 32768)

    # constant block-identity mask: mask[p, c] = (p % 64 == c)
    maskh = np.zeros((128, C), dtype=ml_dtypes.float8_e3m4)
    pp = np.arange(128)
    maskh[pp, pp % C] = ml_dtypes.float8_e3m4(1.0)

    in_maps = []
    for core in range(N_CORES):
        shard = qWf[ROWS * core:ROWS * (core + 1)]                 # (8, 32768)
        sh = shard.reshape(ROWS, 256, 128)                         # [c'l, h, p]
        if MODE == "coltile":
            # h = T*128 + k*8 + ni ; col j = k*128 + T*64 + c'l*8 + ni
            g = sh.reshape(ROWS, 2, 16, 8, 128)                    # [c'l,T,k,ni,p]
            wf_host = np.ascontiguousarray(
                g.transpose(4, 2, 1, 0, 3)).reshape(128, 2048)     # [p,k,T,c'l,ni]
        else:
            # h = k*16 + ni ; col j = k*128 + c'l*16 + ni
            g = sh.reshape(ROWS, 16, 16, 128)                      # [c'l,k,ni,p]
            wf_host = np.ascontiguousarray(
                g.transpose(3, 1, 0, 2)).reshape(128, 2048)        # [p,k,c'l,ni]

        aux = np.zeros((128, AUX_COLS), np.float32)
        aux[0:ROWS, BFT_COL] = bf[ROWS * core:ROWS * (core + 1)]
        aux[0:ROWS, SWF_COL] = swf
        small_c = small.copy()
        small_c[:, AUXBITS0:AUXBITS0 + 2 * AUX_COLS] = aux.view(np.float16)
        in_maps.append({"wf": wf_host, "small": small_c, "cst": maskh})
    return in_maps


def kernel(x, W_heads, a1_heads, a2_heads, W_out, a1_out, a2_out, Wf, bf):
    global _NC_CACHE
    if _NC_CACHE is None:
        _NC_CACHE = _build_nc()
    nc = _NC_CACHE

    in_maps = _make_in_maps(x, W_heads, W_out, Wf, bf)
    global _last_in_maps
    _last_in_maps = in_maps
    res = run_bass_kernel_spmd(nc, in_maps, list(range(N_CORES)))
    outT = np.concatenate([res.results[i]["out"] for i in range(N_CORES)], axis=0)
    return np.ascontiguousarray(outT.T)                            # (32, 64)
